# revision 26
# baseline (speedup 1.0000x reference)
"""Trainium2 Bass kernel for nn_C2fPSA (quaternion C2fPSA block).

Sharding: the 8 (batch, quaternion) slices are independent except for the 6
BatchNorm statistics, which are synced cross-core.  Each core processes one
(b, q) slice of shape [C, 24, 24] in channel-major [C, n=576] layout; all
convs run on the TensorEngine (1x1 convs as matmuls, 3x3 convs as 9 shifted
accumulating matmuls, depthwise 3x3 as diagonal-matrix matmuls).  Attention:
16 heads of dim 16, head channels zero-padded to 32 so QK^T can use 4-way
tile_position row tiling; softmax is computed max-free (scores ~N(0, 0.05));
denominators come from a leading ones-column in the augmented V operand.

Software pipeline: each iteration is split into 8 stages, each ending at a
BatchNorm-stats boundary (attention is its own payload-free stage).  Stage s
of iteration k executes at pipeline step t = k + s, so up to 8 iterations are
in flight.  The 6 BN stat syncs produced in one step (by stages of different
iterations) are merged into TWO small AllGathers per step (early payload
[128, 11, 2] = BN1-4 fired mid-step, late payload [128, 6, 2] = BN5-6 at step
end) — 2 collectives per iteration in steady state instead of 6 — and both
land before their consumers run in the next step, taking collective latency
off the critical path entirely.
"""
import numpy as np

NCORES = 8
P = 128
N = 576          # 24*24 spatial tokens per (b, q) slice
NH2 = 288        # free-dim half (psum bank = 512 f32; halves at +0 / +512)
EPS = 1e-5
MCNTS = [128, 128, 128, 128, 64]   # m-chunk sizes for 576 tokens
NST = 8          # pipeline stages
PAYA = 11        # early payload: bn1:4 bn2:4 bn3:2 bn4:1
PAYB = 6         # late payload:  bn5:2 bn6:4

_CACHE = {}


def _build(repeat=1, no_coll=False, ndev=NCORES):
    import concourse.bacc as bacc
    import concourse.mybir as mybir
    import concourse.tile as tile

    F32 = mybir.dt.float32
    I32 = mybir.dt.int32
    AF = mybir.ActivationFunctionType
    OP = mybir.AluOpType

    nc = bacc.Bacc("TRN2", target_bir_lowering=False, debug=False,
                   num_devices=ndev)
    BF16 = mybir.dt.bfloat16

    # ---------------- DRAM I/O ----------------
    din = {}
    def dram_in(name, shape, dt=None):
        din[name] = nc.dram_tensor(name, list(shape), dt or F32,
                                   kind="ExternalInput")
        return din[name]

    x_d = dram_in("x_s", (512, N), BF16)
    g_d = dram_in("gvec", (P, 1))
    w1_d = dram_in("w1t", (512, 512), BF16)
    wq_d = dram_in("wqt", (256, 512), BF16)
    wk_d = dram_in("wkt", (256, 512), BF16)
    wv_d = dram_in("wvt", (256, 256), BF16)
    wa_d = dram_in("wat_pad", (512, 256), BF16)
    pe_d = dram_in("pe_w", (256, 9))
    wf1_d = dram_in("wf1t", (256, 512), BF16)
    wf2_d = dram_in("wf2t", (512, 256), BF16)
    wec_d = dram_in("wect", (256, 128), BF16)
    wmp_d = dram_in("wmpt", (9, 128, 256), BF16)
    w2_d = dram_in("w2t", (1024, 512), BF16)
    id_d = dram_in("id128", (P, P))
    e4_d = dram_in("e4", (4, P), BF16)
    out_d = nc.dram_tensor("out", [512, N], BF16, kind="ExternalOutput")

    with tile.TileContext(nc) as tc:
        import contextlib
        ctx = contextlib.ExitStack()
        with ctx:
            ctx.enter_context(nc.allow_low_precision(
                reason="float32r matmul inputs; tolerance 2e-2"))
            sb = ctx.enter_context(tc.tile_pool(name="sb", bufs=1))
            est_pool = ctx.enter_context(tc.tile_pool(name="est", bufs=5))
            avsb_pool = ctx.enter_context(tc.tile_pool(name="avsb", bufs=2))
            small = ctx.enter_context(tc.tile_pool(name="small", bufs=3))
            ps_conv = ctx.enter_context(
                tc.tile_pool(name="ps_conv", bufs=2, space="PSUM"))
            ps_tail = ctx.enter_context(
                tc.tile_pool(name="ps_tail", bufs=1, space="PSUM"))
            ps_av = ctx.enter_context(
                tc.tile_pool(name="ps_av", bufs=1, space="PSUM"))
            dram = ctx.enter_context(
                tc.tile_pool(name="dram", bufs=3, space="DRAM"))

            def ld(dst, src):
                nc.sync.dma_start(dst, src)

            id128 = sb.tile([P, P], F32)
            e4t = sb.tile([4, P], BF16)

            # consts
            ones_row = sb.tile([1, P], F32)
            ones_col = sb.tile([P, 1], F32)
            zcb = sb.tile([1, P], BF16)
            zrb = sb.tile([1, NH2], BF16)
            nc.vector.memset(ones_row[:], 1.0)
            nc.vector.memset(ones_col[:], 1.0)
            nc.vector.memset(zcb[:], 0.0)
            nc.vector.memset(zrb[:], 0.0)

            junk_sq = sb.tile([P, N], BF16)
            # ACT table prewarm (exp set stays loaded for the whole kernel)
            junk1 = small.tile([1, 1], F32, tag="junk1")
            nc.scalar.activation(junk1[:], ones_row[0:1, 0:1], AF.Exp)
            rsq_c = sb.tile([P, 4], F32)
            nc.vector.memset(rsq_c[:], float(np.uint32(0x5f3759df).view(np.float32)))

            # ------------- persistent SBUF (iteration-invariant) -------------
            x_sb = sb.tile([P, 4, N], BF16)
            gvec = sb.tile([P, 1], F32)
            w1t = sb.tile([P, 4, 512], BF16)
            wqt = sb.tile([P, 2, 512], BF16)
            wkt = sb.tile([P, 2, 512], BF16)
            wvt = sb.tile([P, 2, 256], BF16)
            wat = sb.tile([P, 4, 256], BF16)
            pew = sb.tile([P, 2, 9], F32)
            wf1t = sb.tile([P, 2, 512], BF16)
            wf2t = sb.tile([P, 4, 256], BF16)
            wect = sb.tile([P, 2, 128], BF16)
            wmpt = sb.tile([P, 9, 256], BF16)
            w2t = sb.tile([P, 8, 512], BF16)
            for kc_ in range(4):
                ld(x_sb[:, kc_, :],
                   x_d[:].rearrange("(a p) f -> p a f", p=P)[:, kc_, :])
            ld(gvec[:], g_d[:])
            ld(w1t[:], w1_d[:].rearrange("(a p) f -> p a f", p=P))
            ld(wqt[:], wq_d[:].rearrange("(a p) f -> p a f", p=P))
            ld(wkt[:], wk_d[:].rearrange("(a p) f -> p a f", p=P))
            ld(wvt[:], wv_d[:].rearrange("(a p) f -> p a f", p=P))
            ld(wat[:], wa_d[:].rearrange("(a p) f -> p a f", p=P))
            ld(pew[:], pe_d[:].rearrange("(a p) f -> p a f", p=P))
            ld(wf1t[:], wf1_d[:].rearrange("(a p) f -> p a f", p=P))
            ld(wf2t[:], wf2_d[:].rearrange("(a p) f -> p a f", p=P))
            ld(wect[:], wec_d[:].rearrange("(a p) f -> p a f", p=P))
            ld(wmpt[:], wmp_d[:].transpose([1, 0, 2]))
            ld(w2t[:], w2_d[:].rearrange("(a p) f -> p a f", p=P))
            ld(id128[:], id_d[:])
            ld(e4t[:], e4_d[:])

            # depthwise positional-conv diagonal weights (iteration-invariant)
            diag_sb = sb.tile([P, 18, P], BF16)
            for mc in range(2):
                for t in range(9):
                    nc.vector.tensor_scalar(
                        diag_sb[:, mc * 9 + t, :], id128[:],
                        pew[:, mc, t:t + 1], None, op0=OP.mult)

            def h3(t):
                """psum tile 3D view [p, 2, 288]."""
                return t[:].rearrange("p (a f) -> p a f", f=512)[:, :, 0:NH2]

            def mm(out, lhsT, rhs, **kw):
                nc.tensor.matmul(out, lhsT, rhs, **kw)

            # ---------- BN stat helpers (merged-collective pipeline) ----------
            def evac_stats(pt, raw3, st, mc, dve=False):
                """Evacuate psum -> raw (bf16/f32 SBUF) and accumulate S, SS.
                S rides the evac copy (ACT, or DVE-with-broadcast-ones for
                stages in the ACT-bound attention window); SS is one fused
                DVE square-reduce."""
                if dve:
                    nc.vector.tensor_tensor_reduce(
                        raw3, h3(pt),
                        ones_col[:, :, None].broadcast_to([P, 2, NH2]),
                        1.0, 0.0, op0=OP.mult, op1=OP.add,
                        accum_out=st[:, mc, 0:1])
                else:
                    nc.scalar.activation(raw3, h3(pt), AF.Copy,
                                         accum_out=st[:, mc, 0:1])
                nc.scalar.activation(
                    junk_sq[:].rearrange("p (a f) -> p a f", f=NH2),
                    h3(pt), AF.Square, accum_out=st[:, mc, 1:2])

            c0 = -1.0 / (N * NCORES)
            c1 = 1.0 / (N * NCORES)

            def stats_pay(pay, off, nchunk, st, gate2=None):
                """st [P,nchunk,2] -> payload slices of the step's pay tile.
                pay0 = -S/(N*8), pay1 = SS/(N*8) + eps/8 so the AllGather sum
                yields -mu_g and E[x^2]_g + eps directly."""
                if gate2 is None:
                    nc.vector.tensor_scalar(pay[:, off:off + nchunk, 0],
                                            st[:, :, 0], c0, None, op0=OP.mult)
                    nc.vector.tensor_scalar(pay[:, off:off + nchunk, 1],
                                            st[:, :, 1], c1, EPS / NCORES,
                                            op0=OP.mult, op1=OP.add)
                else:
                    gb, gb2 = gate2
                    nc.vector.tensor_scalar(pay[:, off:off + nchunk, 0],
                                            st[:, :, 0], gb[:], None,
                                            op0=OP.mult)
                    nc.vector.tensor_scalar(pay[:, off:off + nchunk, 0],
                                            pay[:, off:off + nchunk, 0],
                                            c0, None, op0=OP.mult)
                    nc.vector.tensor_scalar(pay[:, off:off + nchunk, 1],
                                            st[:, :, 1], gb2[:], None,
                                            op0=OP.mult)
                    nc.vector.tensor_scalar(pay[:, off:off + nchunk, 1],
                                            pay[:, off:off + nchunk, 1],
                                            c1, EPS / NCORES,
                                            op0=OP.mult, op1=OP.add)

            def bn_coeff(sums_v, off, nchunk, tag):
                """sums_v [P,17,2] (globally reduced) -> scale r, bias -mu*r."""
                negmu = sums_v[:, off:off + nchunk, 0]
                var = small.tile([P, nchunk], F32, tag=f"var{tag}")
                nc.vector.tensor_tensor(var[:], negmu, negmu, op=OP.mult)
                nc.vector.tensor_tensor(var[:], sums_v[:, off:off + nchunk, 1],
                                        var[:], op=OP.subtract)
                # rsqrt on DVE only (bit-trick seed + 2 Newton iters)
                y0i = small.tile([P, nchunk], I32, tag=f"y0i{tag}")
                nc.vector.tensor_scalar(y0i[:], var[:].bitcast(I32), 1,
                                        None, op0=OP.logical_shift_right)
                nc.vector.tensor_tensor(y0i[:],
                                        rsq_c[:, 0:nchunk].bitcast(I32),
                                        y0i[:], op=OP.subtract)
                r = small.tile([P, nchunk], F32, tag=f"r{tag}")
                ntmp = small.tile([P, nchunk], F32, tag=f"ntmp{tag}")
                cur = y0i[:].bitcast(F32)
                for _it in range(2):
                    nc.vector.tensor_tensor(ntmp[:], cur, cur, op=OP.mult)
                    nc.vector.tensor_tensor(ntmp[:], ntmp[:], var[:],
                                            op=OP.mult)
                    nc.vector.tensor_scalar(ntmp[:], ntmp[:], -0.5, 1.5,
                                            op0=OP.mult, op1=OP.add)
                    nc.vector.tensor_tensor(r[:], cur, ntmp[:], op=OP.mult)
                    cur = r[:]
                nb = small.tile([P, nchunk], F32, tag=f"nb{tag}")
                nc.vector.tensor_tensor(nb[:], negmu, r[:], op=OP.mult)
                return r, nb

            # ===================== pipeline stages =====================
            def s0(S, payA, payB, sA, sB):
                """cv1 convs + BN1 stats (raws stay pre-BN)."""
                y_a = sb.tile([P, 2, N], BF16, bufs=7)
                y_b = sb.tile([P, 2, N], BF16, bufs=7)
                S["y_a"], S["y_b"] = y_a, y_b
                raws = [y_a[:, 0, :], y_a[:, 1, :], y_b[:, 0, :], y_b[:, 1, :]]
                st = small.tile([P, 4, 2], F32, tag="st1")
                for mc in range(4):
                    pt = ps_conv.tile([P, 1024], F32, tag="conv")
                    for nh in range(2):
                        for kc in range(4):
                            mm(pt[:, nh * 512: nh * 512 + NH2],
                               w1t[:, kc, mc * P:(mc + 1) * P],
                               x_sb[:, kc, nh * NH2:(nh + 1) * NH2],
                               start=(kc == 0), stop=(kc == 3))
                    evac_stats(pt, raws[mc].rearrange("p (a f) -> p a f",
                                                      f=NH2), st, mc)
                stats_pay(payA, 0, 4, st)

            def s1a(S, payA, payB, sA, sB):
                """BN1 apply + attention + aproj (no stats payload)."""
                y_a, y_b = S["y_a"], S["y_b"]
                r1, nb1 = bn_coeff(sA, 0, 4, tag=1)
                raws = [y_a[:, 0, :], y_a[:, 1, :], y_b[:, 0, :], y_b[:, 1, :]]
                # apply b-half first (chunks 2,3) so attention starts sooner
                for mc in (2, 3, 0, 1):
                    nc.scalar.activation(raws[mc], raws[mc], AF.Relu,
                                         bias=nb1[:, mc:mc + 1],
                                         scale=r1[:, mc:mc + 1])

                b_pad = sb.tile([P, 2, 676], BF16, bufs=2)
                S["b_pad"] = b_pad
                nc.vector.memset(b_pad[:], 0.0)
                for mc in range(2):
                    nc.vector.tensor_copy(
                        b_pad[:, mc, :].rearrange("p (h w) -> p h w",
                                                  w=26)[:, 1:25, 1:25],
                        y_b[:, mc, :].rearrange("p (h w) -> p h w", w=24))

                q_pad = sb.tile([P, 4, N], BF16, bufs=2)
                k_pad = sb.tile([P, 4, N], BF16, bufs=2)
                v_aug = sb.tile([P, 5, 512], BF16, bufs=2)
                attn_pad = sb.tile([P, 4, N], BF16, bufs=2)
                nc.vector.memset(v_aug[:], 0.0)
                nc.vector.memset(
                    v_aug[:].rearrange("p a (h c) -> p a h c",
                                       c=32)[:, :, :, 0], 1.0)
                # qkv: q_pad / k_pad [P, 4, 576] (head-padded), v^T into v_aug
                for mc in range(4):
                    ptq = ps_conv.tile([P, 1024], F32, tag="conv")
                    for nh in range(2):
                        for kc in range(2):
                            mm(ptq[:, nh * 512: nh * 512 + NH2],
                               wqt[:, kc, mc * P:(mc + 1) * P],
                               y_b[:, kc, nh * NH2:(nh + 1) * NH2],
                               start=(kc == 0), stop=(kc == 1))
                    nc.vector.tensor_copy(
                        q_pad[:, mc, :].rearrange("p (a f) -> p a f", f=NH2),
                        h3(ptq))
                    ptk = ps_conv.tile([P, 1024], F32, tag="conv")
                    for nh in range(2):
                        for kc in range(2):
                            mm(ptk[:, nh * 512: nh * 512 + NH2],
                               wkt[:, kc, mc * P:(mc + 1) * P],
                               y_b[:, kc, nh * NH2:(nh + 1) * NH2],
                               start=(kc == 0), stop=(kc == 1))
                    nc.vector.tensor_copy(
                        k_pad[:, mc, :].rearrange("p (a f) -> p a f", f=NH2),
                        h3(ptk))
                for mcv in range(5):
                    cnt = MCNTS[mcv]
                    ptv = ps_conv.tile([P, 256], F32, tag="conv")
                    for kc in range(2):
                        mm(ptv[0:cnt, :],
                           y_b[:, kc, mcv * P: mcv * P + cnt],
                           wvt[:, kc, :], start=(kc == 0), stop=(kc == 1))
                    nc.vector.tensor_copy(
                        v_aug[0:cnt, mcv, :].rearrange(
                            "p (h c) -> p h c", c=32)[:, :, 1:17],
                        ptv[0:cnt, :].rearrange("p (h d) -> p h d", d=16))

                # per-group attention (4 heads per group, col-tiled AV)
                for g in range(4):
                    av = ps_av.tile([P, 1024], F32, tag="av")
                    for j in range(4):
                        h = 4 * g + j
                        ch, off = h // 4, 32 * (h % 4)
                        for mcv in range(5):
                            cnt = MCNTS[mcv]
                            sp = ps_conv.tile([P, 1024], F32, tag="conv")
                            for nh in range(2):
                                mm(sp[0:cnt, nh * 512: nh * 512 + NH2],
                                   k_pad[off:off + 32, ch,
                                         mcv * P: mcv * P + cnt],
                                   q_pad[off:off + 32, ch,
                                         nh * NH2:(nh + 1) * NH2],
                                   start=True, stop=True,
                                   tile_position=(off, 0))
                            est = est_pool.tile([P, 2, NH2], BF16, tag="est")
                            nc.scalar.activation(
                                est[0:cnt, :, :],
                                sp[0:cnt, :].rearrange(
                                    "p (a f) -> p a f", f=512)[:, :, 0:NH2],
                                AF.Exp, scale=0.25)
                            for nh in range(2):
                                mm(av[off:off + 32,
                                      nh * 512: nh * 512 + NH2],
                                   v_aug[0:cnt, mcv, 32 * h:32 * h + 32],
                                   est[0:cnt, nh, :],
                                   start=(mcv == 0), stop=(mcv == 4),
                                   tile_position=(0, off),
                                   skip_group_check=True)
                    # normalize group: denom rows at 32j (ones-first layout)
                    av_sb = avsb_pool.tile([P, 2, NH2], F32, tag="avsb")
                    nc.vector.tensor_copy(av_sb[:], h3(av))
                    den4 = small.tile([4, 2, NH2], F32, tag="den4")
                    nc.sync.dma_start(den4[:], av_sb[0:P:32, :, :])
                    rec4 = small.tile([4, 2, NH2], BF16, tag="rec4")
                    nc.vector.reciprocal(rec4[:], den4[:])
                    for nh in range(2):
                        rb = ps_av.tile([P, NH2], F32, tag="av")
                        mm(rb[:], e4t[:], rec4[:, nh, :],
                           start=True, stop=True)
                        nc.vector.tensor_tensor(
                            attn_pad[:, g, nh * NH2:(nh + 1) * NH2],
                            av_sb[:, nh, :], rb[:], op=OP.mult)

                # aproj + pe(depthwise) + shortcut -> a_psa
                a_psa = sb.tile([P, 2, N], BF16, bufs=4)
                S["a_psa"] = a_psa
                for mc in range(2):
                    pt = ps_conv.tile([P, 1024], F32, tag="conv")
                    for nh in range(2):
                        for kc in range(4):
                            mm(pt[:, nh * 512: nh * 512 + NH2],
                               wat[:, kc, mc * P:(mc + 1) * P],
                               attn_pad[:, kc, nh * NH2:(nh + 1) * NH2],
                               start=(kc == 0), stop=False)
                        for t in range(9):
                            u, v = t // 3, t % 3
                            win = b_pad[:, mc, :].rearrange(
                                "p (h w) -> p h w", w=26)[
                                :, u + nh * 12: u + nh * 12 + 12, v: v + 24]
                            mm(pt[:, nh * 512: nh * 512 + NH2].rearrange(
                                   "p (h w) -> p h w", w=24),
                               diag_sb[:, mc * 9 + t, :], win,
                               start=False, stop=(t == 8))
                    nc.vector.tensor_tensor(
                        a_psa[:, mc, :].rearrange("p (a f) -> p a f", f=NH2),
                        h3(pt),
                        y_b[:, mc, :].rearrange("p (a f) -> p a f", f=NH2),
                        op=OP.add)

            def s1b(S, payA, payB, sA, sB):
                """ffn1 convs + BN2 stats."""
                a_psa = S["a_psa"]
                h_ffn = sb.tile([P, 4, N], BF16, bufs=3)
                S["h_ffn"] = h_ffn
                st = small.tile([P, 4, 2], F32, tag="st2")
                for mc in range(4):
                    pt = ps_conv.tile([P, 1024], F32, tag="conv")
                    for nh in range(2):
                        for kc in range(2):
                            mm(pt[:, nh * 512: nh * 512 + NH2],
                               wf1t[:, kc, mc * P:(mc + 1) * P],
                               a_psa[:, kc, nh * NH2:(nh + 1) * NH2],
                               start=(kc == 0), stop=(kc == 1))
                    evac_stats(pt, h_ffn[:, mc, :].rearrange(
                        "p (a f) -> p a f", f=NH2), st, mc)
                stats_pay(payA, 4, 4, st)

            def s2(S, payA, payB, sA, sB):
                """BN2 apply + ffn2 convs + BN3 stats."""
                h_ffn = S["h_ffn"]
                r2, nb2 = bn_coeff(sA, 4, 4, tag=2)
                for mc in range(4):
                    buf = h_ffn[:, mc, :]
                    nc.scalar.activation(buf, buf, AF.Relu,
                                         bias=nb2[:, mc:mc + 1],
                                         scale=r2[:, mc:mc + 1])
                f_tmp = sb.tile([P, 2, N], BF16, bufs=3)
                S["f_tmp"] = f_tmp
                st = small.tile([P, 2, 2], F32, tag="st3")
                for mc in range(2):
                    pt = ps_tail.tile([P, 1024], F32, tag="tail")
                    for nh in range(2):
                        for kc in range(4):
                            mm(pt[:, nh * 512: nh * 512 + NH2],
                               wf2t[:, kc, mc * P:(mc + 1) * P],
                               h_ffn[:, kc, nh * NH2:(nh + 1) * NH2],
                               start=(kc == 0), stop=(kc == 3))
                    evac_stats(pt, f_tmp[:, mc, :].rearrange(
                        "p (a f) -> p a f", f=NH2), st, mc)
                stats_pay(payA, 8, 2, st)

            def s3(S, payA, payB, sA, sB):
                """BN3 apply + psa shortcut + ec conv + BN4 stats."""
                f_tmp, a_psa = S["f_tmp"], S["a_psa"]
                r3, nb3 = bn_coeff(sA, 8, 2, tag=3)
                p_sb = sb.tile([P, 2, N], BF16, bufs=3)
                S["p_sb"] = p_sb
                for mc in range(2):
                    buf = f_tmp[:, mc, :]
                    nc.scalar.activation(buf, buf, AF.Identity,
                                         bias=nb3[:, mc:mc + 1],
                                         scale=r3[:, mc:mc + 1])
                    nc.vector.tensor_tensor(p_sb[:, mc, :], f_tmp[:, mc, :],
                                            a_psa[:, mc, :], op=OP.add)
                e_sb = sb.tile([P, N], F32, bufs=2)
                S["e_sb"] = e_sb
                st = small.tile([P, 1, 2], F32, tag="st4")
                ec_pt = ps_tail.tile([P, 1024], F32, tag="tail")
                for nh in range(2):
                    for kc in range(2):
                        mm(ec_pt[:, nh * 512: nh * 512 + NH2],
                           wect[:, kc, :],
                           p_sb[:, kc, nh * NH2:(nh + 1) * NH2],
                           start=(kc == 0), stop=(kc == 1))
                evac_stats(ec_pt, e_sb[:].rearrange("p (a f) -> p a f",
                                                    f=NH2), st, 0)
                stats_pay(payA, 10, 1, st)

            def s4(S, payA, payB, sA, sB):
                """BN4 apply + sigmoid gate + mproj convs + gated BN5 stats."""
                e_sb = S["e_sb"]
                r4, nb4 = bn_coeff(sA, 10, 1, tag=4)
                nc.scalar.activation(e_sb[:], e_sb[:], AF.Relu,
                                     bias=nb4[:, 0:1], scale=r4[:, 0:1])
                # gate = sigmoid(sum(e * g) / sqrt(128*576))
                acc_e = small.tile([P, 1], F32, tag="acc_e")
                nc.scalar.activation(junk_sq[:, 0:N], e_sb[:], AF.Copy,
                                     scale=gvec[:], accum_out=acc_e[:])
                gd_ps = ps_tail.tile([1, 1], F32, tag="tail")
                nc.tensor.matmul(gd_ps[:], ones_col[:], acc_e[:],
                                 start=True, stop=True)
                sg = small.tile([1, 1], F32, tag="sg")
                nc.scalar.activation(sg[:], gd_ps[:], AF.Exp,
                                     scale=-1.0 / float(np.sqrt(128.0 * N)))
                sg1 = small.tile([1, 1], F32, tag="sg1")
                nc.vector.tensor_scalar(sg1[:], sg[:], 1.0, None, op0=OP.add)
                grec = small.tile([1, 1], F32, tag="grec")
                nc.vector.reciprocal(grec[:], sg1[:])
                gb_ps = ps_tail.tile([P, 1], F32, tag="tail")
                nc.tensor.matmul(gb_ps[:], ones_row[:], grec[:],
                                 start=True, stop=True)
                gb = small.tile([P, 1], F32, tag="gb")
                nc.vector.tensor_copy(gb[:], gb_ps[:])
                gb2 = small.tile([P, 1], F32, tag="gb2")
                nc.vector.tensor_tensor(gb2[:], gb[:], gb[:], op=OP.mult)
                S["gb"], S["gb2"] = gb, gb2

                # e_pad + mproj (gate folded into BN via gated stats)
                e_pad = sb.tile([P, 676], BF16, bufs=2)
                nc.vector.memset(e_pad[:], 0.0)
                nc.vector.tensor_copy(
                    e_pad[:].rearrange("p (h w) -> p h w", w=26)[:, 1:25, 1:25],
                    e_sb[:].rearrange("p (h w) -> p h w", w=24))
                m_sb = sb.tile([P, 2, N], BF16, bufs=3)
                S["m_sb"] = m_sb
                st = small.tile([P, 2, 2], F32, tag="st5")
                for mc in range(2):
                    pt = ps_tail.tile([P, 1024], F32, tag="tail")
                    for nh in range(2):
                        for t in range(9):
                            u, v = t // 3, t % 3
                            win = e_pad[:].rearrange("p (h w) -> p h w",
                                                     w=26)[
                                :, u + nh * 12: u + nh * 12 + 12, v: v + 24]
                            mm(pt[:, nh * 512: nh * 512 + NH2].rearrange(
                                   "p (h w) -> p h w", w=24),
                               wmpt[:, t, mc * P:(mc + 1) * P], win,
                               start=(t == 0), stop=(t == 8))
                    evac_stats(pt, m_sb[:, mc, :].rearrange(
                        "p (a f) -> p a f", f=NH2), st, mc)
                stats_pay(payB, 0, 2, st, gate2=(gb, gb2))

            def s5(S, payA, payB, sA, sB):
                """BN5 apply (gated) + full cv2 convs + BN6 stats."""
                m_sb, gb = S["m_sb"], S["gb"]
                r5, nb5 = bn_coeff(sB, 0, 2, tag=5)
                r5g = small.tile([P, 2], F32, tag="r5g")
                nc.vector.tensor_scalar(r5g[:], r5[:], gb[:], None,
                                        op0=OP.mult)
                for mc in range(2):
                    buf = m_sb[:, mc, :]
                    nc.scalar.activation(buf, buf, AF.Relu,
                                         bias=nb5[:, mc:mc + 1],
                                         scale=r5g[:, mc:mc + 1])
                y_a, y_b, p_sb = S["y_a"], S["y_b"], S["p_sb"]
                cat2 = [y_a[:, 0, :], y_a[:, 1, :], y_b[:, 0, :], y_b[:, 1, :],
                        p_sb[:, 0, :], p_sb[:, 1, :],
                        m_sb[:, 0, :], m_sb[:, 1, :]]
                out_sb = sb.tile([P, 4, N], BF16, bufs=3)
                S["out_sb"] = out_sb
                st = small.tile([P, 4, 2], F32, tag="st6")
                for mc in range(4):
                    pt = ps_tail.tile([P, 1024], F32, tag="tail")
                    for nh in range(2):
                        for kc in range(8):
                            mm(pt[:, nh * 512: nh * 512 + NH2],
                               w2t[:, kc, mc * P:(mc + 1) * P],
                               cat2[kc][:, nh * NH2:(nh + 1) * NH2],
                               start=(kc == 0), stop=(kc == 7))
                    evac_stats(pt, out_sb[:, mc, :].rearrange(
                        "p (a f) -> p a f", f=NH2), st, mc)
                stats_pay(payB, 2, 4, st)

            def s6(S, payA, payB, sA, sB):
                """BN6 apply + output DMA."""
                out_sb = S["out_sb"]
                r6, nb6 = bn_coeff(sB, 2, 4, tag=6)
                for mc in range(4):
                    buf = out_sb[:, mc, :]
                    nc.scalar.activation(buf, buf, AF.Relu,
                                         bias=nb6[:, mc:mc + 1],
                                         scale=r6[:, mc:mc + 1])
                    nc.sync.dma_start(
                        out_d[:].rearrange("(a p) f -> p a f", p=P)[:, mc, :],
                        buf)
                S.clear()

            stages = [s0, s1a, s1b, s2, s3, s4, s5, s6]
            a_producers = [0, 2, 3, 4]     # s0, s1b, s2, s3 fill payA
            b_producers = [1, 5, 6, 7]     # s1a (no pay), s4, s5, s6

            def do_coll(pay, nch, tag):
                bin_ = dram.tile([P, nch, 2], F32, tag=f"ccin{tag}",
                                 name=f"bin{tag}")
                bout = dram.tile([NCORES, P, nch, 2], F32, tag=f"ccout{tag}",
                                 name=f"bout{tag}")
                nc.sync.dma_start(bin_[:], pay[:])
                nc.gpsimd.collective_compute(
                    "AllGather", OP.bypass,
                    replica_groups=[list(range(NCORES))],
                    ins=[bin_[:].opt()], outs=[bout[:].opt()])
                gat = small.tile([P, NCORES, nch, 2], F32, tag=f"gat{tag}",
                                 name=f"gat{tag}")
                for rk in range(NCORES):
                    nc.sync.dma_start(gat[:, rk, :, :], bout[rk])
                sums = small.tile([P, nch, 2, 1], F32, tag=f"sums{tag}",
                                  name=f"sums{tag}")
                nc.vector.reduce_sum(sums[:],
                                     gat[:].transpose([0, 2, 3, 1]),
                                     axis=mybir.AxisListType.X)
                return sums[:, :, :, 0]

            # ===================== pipeline driver =====================
            states = [dict() for _ in range(repeat)]
            sums_of_step = {}
            for t in range(repeat + NST - 1):
                payA = small.tile([P, PAYA, 2], F32, tag="payA")
                payB = small.tile([P, PAYB, 2], F32, tag="payB")
                nc.vector.memset(payA[:], 0.0)
                nc.vector.memset(payB[:], 0.0)
                sA, sB = sums_of_step.pop(t - 1, (None, None))
                for s in a_producers:
                    k = t - s
                    if 0 <= k < repeat:
                        stages[s](states[k], payA, payB, sA, sB)
                cA = payA if no_coll else do_coll(payA, PAYA, "A")
                for s in b_producers:
                    k = t - s
                    if 0 <= k < repeat:
                        stages[s](states[k], payA, payB, sA, sB)
                cB = payB if no_coll else do_coll(payB, PAYB, "B")
                sums_of_step[t] = (cA, cB)

    nc.compile()
    return nc


def get_nc(repeat=1, no_coll=False, ndev=NCORES):
    key = f"nc{repeat}_{no_coll}_{ndev}"
    if key not in _CACHE:
        _CACHE[key] = _build(repeat, no_coll, ndev)
    return _CACHE[key]


def host_prep(inputs):
    """Fold modulus*cos(phase) weights, build padded/transposed layouts and
    the 8 per-core input maps."""
    def w(m, p):
        return (np.asarray(inputs[m], np.float32)
                * np.cos(np.asarray(inputs[p], np.float32)))

    x = np.asarray(inputs["x"], np.float32)          # (2, 512, 4, 24, 24)
    guide = np.asarray(inputs["guide"], np.float32)  # (2, 512)
    B, C1, Qd = x.shape[0], x.shape[1], x.shape[2]

    w1 = w("cv1_m", "cv1_p")[:, :, 0, 0]             # (512, 512) [co, ci]
    wqkv = w("qkv_m", "qkv_p")[:, :, 0, 0]           # (768, 256)
    wq, wk, wv = wqkv[0:256], wqkv[256:512], wqkv[512:768]
    wa = w("aproj_m", "aproj_p")[:, :, 0, 0]         # (256, 256)
    pe = w("pe_m", "pe_p")[:, 0, :, :].reshape(256, 9)
    wf1 = w("ffn1_m", "ffn1_p")[:, :, 0, 0]          # (512, 256)
    wf2 = w("ffn2_m", "ffn2_p")[:, :, 0, 0]          # (256, 512)
    wec = w("ec_m", "ec_p")[:, :, 0, 0]              # (128, 256)
    wmp = w("mproj_m", "mproj_p")                    # (256, 128, 3, 3)
    w2 = w("cv2_m", "cv2_p")[:, :, 0, 0]             # (512, 1024)
    gl_w = np.asarray(inputs["gl_w"], np.float32)
    gl_b = np.asarray(inputs["gl_b"], np.float32)
    gfull = guide @ gl_w.T + gl_b                    # (2, 128)

    # q/k head-padded [ci, co'=512]: col 32h+d = W[16h+d, ci], d<16
    def pad_qk(wm):
        out = np.zeros((256, 512), np.float32)
        for h in range(16):
            out[:, 32 * h:32 * h + 16] = wm[16 * h:16 * h + 16, :].T
        return out

    wqt = pad_qk(wq)
    wkt = pad_qk(wk)
    # aproj with padded ci' (ones-first attn layout: channel' 32h+1+d)
    wat_pad = np.zeros((512, 256), np.float32)
    for h in range(16):
        wat_pad[32 * h + 1:32 * h + 17, :] = wa[:, 16 * h:16 * h + 16].T
    e4 = np.zeros((4, 128), np.float32)
    for j in range(4):
        e4[j, 32 * j + 1:32 * j + 17] = 1.0

    import ml_dtypes
    bf16 = ml_dtypes.bfloat16
    shared = {
        "w1t": np.ascontiguousarray(w1.T).astype(bf16),
        "wqt": wqt.astype(bf16), "wkt": wkt.astype(bf16),
        "wvt": np.ascontiguousarray(wv.T).astype(bf16),
        "wat_pad": wat_pad.astype(bf16),
        "pe_w": pe,
        "wf1t": np.ascontiguousarray(wf1.T).astype(bf16),
        "wf2t": np.ascontiguousarray(wf2.T).astype(bf16),
        "wect": np.ascontiguousarray(wec.T).astype(bf16),
        "wmpt": np.ascontiguousarray(
            wmp.transpose(2, 3, 1, 0).reshape(9, 128, 256)).astype(bf16),
        "w2t": np.ascontiguousarray(w2.T).astype(bf16),
        "id128": np.eye(128, dtype=np.float32),
        "e4": e4.astype(bf16),
    }
    in_maps = []
    for core in range(NCORES):
        b, q = core // Qd, core % Qd
        m = dict(shared)
        m["x_s"] = np.ascontiguousarray(x[b, :, q].reshape(C1, N)).astype(bf16)
        m["gvec"] = np.ascontiguousarray(gfull[b].reshape(P, 1))
        in_maps.append(m)
    return in_maps, (B, Qd)


def get_runner(repeat=1, no_coll=False):
    """Cached sharded jitted executable over the 8 axon cores, mirroring
    bass2jax.run_bass_via_pjrt (which re-traces on every call)."""
    rkey = f"runner{repeat}_{no_coll}"
    if rkey in _CACHE:
        return _CACHE[rkey]
    import jax
    import numpy as _np
    from jax.sharding import Mesh, PartitionSpec
    from jax.experimental.shard_map import shard_map
    import concourse.mybir as mybir
    from concourse.bass2jax import (_bass_exec_p, partition_id_tensor,
                                    install_neuronx_cc_hook)

    nc = get_nc(repeat, no_coll)
    install_neuronx_cc_hook()
    partition_name = (nc.partition_id_tensor.name
                      if nc.partition_id_tensor else None)
    in_names, out_names, out_avals, zero_outs = [], [], [], []
    for alloc in nc.m.functions[0].allocations:
        if not isinstance(alloc, mybir.MemoryLocationSet):
            continue
        name = alloc.memorylocations[0].name
        if alloc.kind == "ExternalInput":
            if name != partition_name:
                in_names.append(name)
        elif alloc.kind == "ExternalOutput":
            shape = tuple(alloc.tensor_shape)
            dtype = mybir.dt.np(alloc.dtype)
            out_names.append(name)
            out_avals.append(jax.core.ShapedArray(shape, dtype))
            zero_outs.append(_np.zeros(shape, dtype))
    n_params, n_outs = len(in_names), len(out_avals)
    all_in_names = list(in_names) + list(out_names)
    if partition_name is not None:
        all_in_names.append(partition_name)
    donate = tuple(range(n_params, n_params + n_outs))

    def _body(*args):
        operands = list(args)
        if partition_name is not None:
            operands.append(partition_id_tensor())
        outs = _bass_exec_p.bind(
            *operands,
            out_avals=tuple(out_avals),
            in_names=tuple(all_in_names),
            out_names=tuple(out_names),
            lowering_input_output_aliases=(),
            sim_require_finite=True,
            sim_require_nnan=True,
            nc=nc,
        )
        return tuple(outs)

    devices = jax.devices()[:NCORES]
    mesh = Mesh(_np.asarray(devices), ("core",))
    in_specs = (PartitionSpec("core"),) * (n_params + n_outs)
    out_specs = (PartitionSpec("core"),) * n_outs
    sharded = jax.jit(
        shard_map(_body, mesh=mesh, in_specs=in_specs, out_specs=out_specs,
                  check_rep=False),
        donate_argnums=donate, keep_unused=True)
    runner = {
        "fn": sharded, "mesh": mesh, "in_names": in_names,
        "out_names": out_names, "out_avals": out_avals,
        "zero_outs": zero_outs, "n_params": n_params,
    }
    _CACHE[rkey] = runner
    return runner


def run_cores(in_maps):
    import numpy as _np
    r = get_runner()
    concat_in = [
        _np.concatenate([_np.asarray(in_maps[c][name])[None]
                         for c in range(NCORES)], axis=0).reshape(
            NCORES * in_maps[0][name].shape[0], *in_maps[0][name].shape[1:])
        for name in r["in_names"]]
    concat_zeros = [
        _np.zeros((NCORES * z.shape[0], *z.shape[1:]), z.dtype)
        for z in r["zero_outs"]]
    out_arrs = r["fn"](*concat_in, *concat_zeros)
    outs = []
    for c in range(NCORES):
        outs.append({
            name: _np.asarray(out_arrs[i]).reshape(
                NCORES, *r["out_avals"][i].shape)[c]
            for i, name in enumerate(r["out_names"])})
    return outs


def kernel(**inputs):
    in_maps, (B, Qd) = host_prep(inputs)
    results = run_cores(in_maps)
    out = np.zeros((B, 512, Qd, 24, 24), np.float32)
    for core in range(NCORES):
        b, q = core // Qd, core % Qd
        out[b, :, q] = np.asarray(results[core]["out"],
                                  np.float32).reshape(512, 24, 24)
    return out


# revision 27
# speedup vs baseline: 1.5734x; 1.5734x over previous
"""Trainium2 Bass kernel for nn_C2fPSA (quaternion C2fPSA block).

Sharding: the 8 (batch, quaternion) slices are independent except for the 6
BatchNorm statistics, which are synced cross-core.  Each core processes one
(b, q) slice of shape [C, 24, 24] in channel-major [C, n=576] layout; all
convs run on the TensorEngine (1x1 convs as matmuls, 3x3 convs as 9 shifted
accumulating matmuls, depthwise 3x3 as diagonal-matrix matmuls).  Attention:
16 heads of dim 16, head channels zero-padded to 32 so QK^T can use 4-way
tile_position row tiling; softmax is computed max-free (scores ~N(0, 0.05));
denominators come from a leading ones-column in the augmented V operand.

Software pipeline: each iteration is split into 8 stages, each ending at a
BatchNorm-stats boundary (attention is its own payload-free stage).  Stage s
of iteration k executes at pipeline step t = k + s, so up to 8 iterations are
in flight.  The 6 BN stat syncs produced in one step (by stages of different
iterations) are merged into TWO small AllGathers per step (early payload
[128, 11, 2] = BN1-4 fired mid-step, late payload [128, 6, 2] = BN5-6 at step
end) — 2 collectives per iteration in steady state instead of 6 — and both
land before their consumers run in the next step, taking collective latency
off the critical path entirely.
"""
import numpy as np

NCORES = 8
P = 128
N = 576          # 24*24 spatial tokens per (b, q) slice
NH2 = 288        # free-dim half (psum bank = 512 f32; halves at +0 / +512)
EPS = 1e-5
MCNTS = [128, 128, 128, 128, 64]   # m-chunk sizes for 576 tokens
NST = 8          # pipeline stages
PAYA = 11        # early payload: bn1:4 bn2:4 bn3:2 bn4:1
PAYB = 6         # late payload:  bn5:2 bn6:4

_CACHE = {}


def _build(repeat=1, no_coll=False, ndev=NCORES):
    import concourse.bacc as bacc
    import concourse.mybir as mybir
    import concourse.tile as tile

    F32 = mybir.dt.float32
    I32 = mybir.dt.int32
    AF = mybir.ActivationFunctionType
    OP = mybir.AluOpType

    nc = bacc.Bacc("TRN2", target_bir_lowering=False, debug=False,
                   num_devices=ndev)
    BF16 = mybir.dt.bfloat16

    # ---------------- DRAM I/O ----------------
    din = {}
    def dram_in(name, shape, dt=None):
        din[name] = nc.dram_tensor(name, list(shape), dt or F32,
                                   kind="ExternalInput")
        return din[name]

    x_d = dram_in("x_s", (512, N), BF16)
    g_d = dram_in("gvec", (P, 1))
    w1_d = dram_in("w1t", (512, 512), BF16)
    wq_d = dram_in("wqt", (256, 512), BF16)
    wk_d = dram_in("wkt", (256, 512), BF16)
    wv_d = dram_in("wvt", (256, 256), BF16)
    wa_d = dram_in("wat_pad", (512, 256), BF16)
    pe_d = dram_in("pe_w", (256, 9))
    wf1_d = dram_in("wf1t", (256, 512), BF16)
    wf2_d = dram_in("wf2t", (512, 256), BF16)
    wec_d = dram_in("wect", (256, 128), BF16)
    wmp_d = dram_in("wmpt", (9, 128, 256), BF16)
    w2_d = dram_in("w2t", (1024, 512), BF16)
    id_d = dram_in("id128", (P, P))
    e4_d = dram_in("e4", (4, P), BF16)
    out_d = nc.dram_tensor("out", [512, N], BF16, kind="ExternalOutput")

    with tile.TileContext(nc) as tc:
        import contextlib
        ctx = contextlib.ExitStack()
        with ctx:
            ctx.enter_context(nc.allow_low_precision(
                reason="float32r matmul inputs; tolerance 2e-2"))
            sb = ctx.enter_context(tc.tile_pool(name="sb", bufs=1))
            est_pool = ctx.enter_context(tc.tile_pool(name="est", bufs=4))
            avsb_pool = ctx.enter_context(tc.tile_pool(name="avsb", bufs=2))
            small = ctx.enter_context(tc.tile_pool(name="small", bufs=3))
            ps_conv = ctx.enter_context(
                tc.tile_pool(name="ps_conv", bufs=2, space="PSUM"))
            ps_tail = ctx.enter_context(
                tc.tile_pool(name="ps_tail", bufs=1, space="PSUM"))
            ps_av = ctx.enter_context(
                tc.tile_pool(name="ps_av", bufs=1, space="PSUM"))
            dram = ctx.enter_context(
                tc.tile_pool(name="dram", bufs=3, space="DRAM"))

            def ld(dst, src):
                nc.sync.dma_start(dst, src)

            id128 = sb.tile([P, P], F32)
            e4t = sb.tile([4, P], BF16)

            # consts
            ones_row = sb.tile([1, P], F32)
            ones_col = sb.tile([P, 1], F32)
            zcb = sb.tile([1, P], BF16)
            zrb = sb.tile([1, NH2], BF16)
            nc.vector.memset(ones_row[:], 1.0)
            nc.vector.memset(ones_col[:], 1.0)
            nc.vector.memset(zcb[:], 0.0)
            nc.vector.memset(zrb[:], 0.0)

            junk_sq = sb.tile([P, N], BF16)
            # ACT table prewarm (exp set stays loaded for the whole kernel)
            junk1 = small.tile([1, 1], F32, tag="junk1")
            nc.scalar.activation(junk1[:], ones_row[0:1, 0:1], AF.Exp)
            rsq_c = sb.tile([P, 4], F32)
            nc.vector.memset(rsq_c[:], float(np.uint32(0x5f3759df).view(np.float32)))

            # ------------- persistent SBUF (iteration-invariant) -------------
            x_sb = sb.tile([P, 4, N], BF16)
            gvec = sb.tile([P, 1], F32)
            w1t = sb.tile([P, 4, 512], BF16)
            wqt = sb.tile([P, 2, 512], BF16)
            wkt = sb.tile([P, 2, 512], BF16)
            wvt = sb.tile([P, 2, 256], BF16)
            wat = sb.tile([P, 4, 256], BF16)
            pew = sb.tile([P, 2, 9], F32)
            wf1t = sb.tile([P, 2, 512], BF16)
            wf2t = sb.tile([P, 4, 256], BF16)
            wect = sb.tile([P, 2, 128], BF16)
            wmpt = sb.tile([P, 9, 256], BF16)
            w2t = sb.tile([P, 8, 512], BF16)
            for kc_ in range(4):
                ld(x_sb[:, kc_, :],
                   x_d[:].rearrange("(a p) f -> p a f", p=P)[:, kc_, :])
            ld(gvec[:], g_d[:])
            ld(w1t[:], w1_d[:].rearrange("(a p) f -> p a f", p=P))
            ld(wqt[:], wq_d[:].rearrange("(a p) f -> p a f", p=P))
            ld(wkt[:], wk_d[:].rearrange("(a p) f -> p a f", p=P))
            ld(wvt[:], wv_d[:].rearrange("(a p) f -> p a f", p=P))
            ld(wat[:], wa_d[:].rearrange("(a p) f -> p a f", p=P))
            ld(pew[:], pe_d[:].rearrange("(a p) f -> p a f", p=P))
            ld(wf1t[:], wf1_d[:].rearrange("(a p) f -> p a f", p=P))
            ld(wf2t[:], wf2_d[:].rearrange("(a p) f -> p a f", p=P))
            ld(wect[:], wec_d[:].rearrange("(a p) f -> p a f", p=P))
            ld(wmpt[:], wmp_d[:].transpose([1, 0, 2]))
            ld(w2t[:], w2_d[:].rearrange("(a p) f -> p a f", p=P))
            ld(id128[:], id_d[:])
            ld(e4t[:], e4_d[:])

            # depthwise positional-conv diagonal weights (iteration-invariant)
            diag_sb = sb.tile([P, 18, P], BF16)
            for mc in range(2):
                for t in range(9):
                    nc.vector.tensor_scalar(
                        diag_sb[:, mc * 9 + t, :], id128[:],
                        pew[:, mc, t:t + 1], None, op0=OP.mult)

            def h3(t):
                """psum tile 3D view [p, 2, 288]."""
                return t[:].rearrange("p (a f) -> p a f", f=512)[:, :, 0:NH2]

            def mm(out, lhsT, rhs, **kw):
                nc.tensor.matmul(out, lhsT, rhs, **kw)

            # ---------- BN stat helpers (merged-collective pipeline) ----------
            def evac_stats(pt, raw3, st, mc, dve=False):
                """Evacuate psum -> raw (bf16/f32 SBUF) and accumulate S, SS.
                S rides the evac copy (ACT, or DVE-with-broadcast-ones for
                stages in the ACT-bound attention window); SS is one fused
                DVE square-reduce."""
                if dve:
                    nc.vector.tensor_tensor_reduce(
                        raw3, h3(pt),
                        ones_col[:, :, None].broadcast_to([P, 2, NH2]),
                        1.0, 0.0, op0=OP.mult, op1=OP.add,
                        accum_out=st[:, mc, 0:1])
                else:
                    nc.scalar.activation(raw3, h3(pt), AF.Copy,
                                         accum_out=st[:, mc, 0:1])
                nc.scalar.activation(
                    junk_sq[:].rearrange("p (a f) -> p a f", f=NH2),
                    h3(pt), AF.Square, accum_out=st[:, mc, 1:2])

            c0 = -1.0 / (N * NCORES)
            c1 = 1.0 / (N * NCORES)

            def stats_pay(pay, off, nchunk, st, gate2=None):
                """st [P,nchunk,2] -> payload slices of the step's pay tile.
                pay0 = -S/(N*8), pay1 = SS/(N*8) + eps/8 so the AllGather sum
                yields -mu_g and E[x^2]_g + eps directly."""
                if gate2 is None:
                    nc.vector.tensor_scalar(pay[:, off:off + nchunk, 0],
                                            st[:, :, 0], c0, None, op0=OP.mult)
                    nc.vector.tensor_scalar(pay[:, off:off + nchunk, 1],
                                            st[:, :, 1], c1, EPS / NCORES,
                                            op0=OP.mult, op1=OP.add)
                else:
                    gb, gb2 = gate2
                    nc.vector.tensor_scalar(pay[:, off:off + nchunk, 0],
                                            st[:, :, 0], gb[:], None,
                                            op0=OP.mult)
                    nc.vector.tensor_scalar(pay[:, off:off + nchunk, 0],
                                            pay[:, off:off + nchunk, 0],
                                            c0, None, op0=OP.mult)
                    nc.vector.tensor_scalar(pay[:, off:off + nchunk, 1],
                                            st[:, :, 1], gb2[:], None,
                                            op0=OP.mult)
                    nc.vector.tensor_scalar(pay[:, off:off + nchunk, 1],
                                            pay[:, off:off + nchunk, 1],
                                            c1, EPS / NCORES,
                                            op0=OP.mult, op1=OP.add)

            def bn_coeff(sums_v, off, nchunk, tag):
                """sums_v [P,17,2] (globally reduced) -> scale r, bias -mu*r."""
                negmu = sums_v[:, off:off + nchunk, 0]
                var = small.tile([P, nchunk], F32, tag=f"var{tag}")
                nc.vector.tensor_tensor(var[:], negmu, negmu, op=OP.mult)
                nc.vector.tensor_tensor(var[:], sums_v[:, off:off + nchunk, 1],
                                        var[:], op=OP.subtract)
                # rsqrt on DVE only (bit-trick seed + 2 Newton iters)
                y0i = small.tile([P, nchunk], I32, tag=f"y0i{tag}")
                nc.vector.tensor_scalar(y0i[:], var[:].bitcast(I32), 1,
                                        None, op0=OP.logical_shift_right)
                nc.vector.tensor_tensor(y0i[:],
                                        rsq_c[:, 0:nchunk].bitcast(I32),
                                        y0i[:], op=OP.subtract)
                r = small.tile([P, nchunk], F32, tag=f"r{tag}")
                ntmp = small.tile([P, nchunk], F32, tag=f"ntmp{tag}")
                cur = y0i[:].bitcast(F32)
                for _it in range(2):
                    nc.vector.tensor_tensor(ntmp[:], cur, cur, op=OP.mult)
                    nc.vector.tensor_tensor(ntmp[:], ntmp[:], var[:],
                                            op=OP.mult)
                    nc.vector.tensor_scalar(ntmp[:], ntmp[:], -0.5, 1.5,
                                            op0=OP.mult, op1=OP.add)
                    nc.vector.tensor_tensor(r[:], cur, ntmp[:], op=OP.mult)
                    cur = r[:]
                nb = small.tile([P, nchunk], F32, tag=f"nb{tag}")
                nc.vector.tensor_tensor(nb[:], negmu, r[:], op=OP.mult)
                return r, nb

            # ===================== pipeline stages =====================
            def s0(S, payA, payB, sA, sB):
                """cv1 convs + BN1 stats (raws stay pre-BN)."""
                y_a = sb.tile([P, 2, N], BF16, bufs=7)
                y_b = sb.tile([P, 2, N], BF16, bufs=7)
                S["y_a"], S["y_b"] = y_a, y_b
                raws = [y_a[:, 0, :], y_a[:, 1, :], y_b[:, 0, :], y_b[:, 1, :]]
                st = small.tile([P, 4, 2], F32, tag="st1")
                for mc in range(4):
                    pt = ps_conv.tile([P, 1024], F32, tag="conv")
                    for nh in range(2):
                        for kc in range(4):
                            mm(pt[:, nh * 512: nh * 512 + NH2],
                               w1t[:, kc, mc * P:(mc + 1) * P],
                               x_sb[:, kc, nh * NH2:(nh + 1) * NH2],
                               start=(kc == 0), stop=(kc == 3))
                    evac_stats(pt, raws[mc].rearrange("p (a f) -> p a f",
                                                      f=NH2), st, mc)
                stats_pay(payA, 0, 4, st)

            def s1a(S, payA, payB, sA, sB):
                """BN1 apply + attention + aproj (no stats payload)."""
                y_a, y_b = S["y_a"], S["y_b"]
                r1, nb1 = bn_coeff(sA, 0, 4, tag=1)
                raws = [y_a[:, 0, :], y_a[:, 1, :], y_b[:, 0, :], y_b[:, 1, :]]
                # apply b-half first (chunks 2,3) so attention starts sooner
                for mc in (2, 3, 0, 1):
                    nc.scalar.activation(raws[mc], raws[mc], AF.Relu,
                                         bias=nb1[:, mc:mc + 1],
                                         scale=r1[:, mc:mc + 1])

                b_pad = sb.tile([P, 2, 676], BF16, bufs=2)
                S["b_pad"] = b_pad
                nc.vector.memset(b_pad[:], 0.0)
                for mc in range(2):
                    nc.vector.tensor_copy(
                        b_pad[:, mc, :].rearrange("p (h w) -> p h w",
                                                  w=26)[:, 1:25, 1:25],
                        y_b[:, mc, :].rearrange("p (h w) -> p h w", w=24))

                q_pad = sb.tile([P, 4, N], BF16, bufs=2)
                k_pad = sb.tile([P, 4, N], BF16, bufs=2)
                v_aug = sb.tile([P, 5, 512], BF16, bufs=2)
                attn_pad = sb.tile([P, 4, N], BF16, bufs=2)
                nc.vector.memset(v_aug[:], 0.0)
                nc.vector.memset(
                    v_aug[:].rearrange("p a (h c) -> p a h c",
                                       c=32)[:, :, :, 0], 1.0)
                # qkv: q_pad / k_pad [P, 4, 576] (head-padded), v^T into v_aug
                for mc in range(4):
                    ptq = ps_conv.tile([P, 1024], F32, tag="conv")
                    for nh in range(2):
                        for kc in range(2):
                            mm(ptq[:, nh * 512: nh * 512 + NH2],
                               wqt[:, kc, mc * P:(mc + 1) * P],
                               y_b[:, kc, nh * NH2:(nh + 1) * NH2],
                               start=(kc == 0), stop=(kc == 1))
                    nc.vector.tensor_copy(
                        q_pad[:, mc, :].rearrange("p (a f) -> p a f", f=NH2),
                        h3(ptq))
                    ptk = ps_conv.tile([P, 1024], F32, tag="conv")
                    for nh in range(2):
                        for kc in range(2):
                            mm(ptk[:, nh * 512: nh * 512 + NH2],
                               wkt[:, kc, mc * P:(mc + 1) * P],
                               y_b[:, kc, nh * NH2:(nh + 1) * NH2],
                               start=(kc == 0), stop=(kc == 1))
                    nc.vector.tensor_copy(
                        k_pad[:, mc, :].rearrange("p (a f) -> p a f", f=NH2),
                        h3(ptk))
                for mcv in range(5):
                    cnt = MCNTS[mcv]
                    ptv = ps_conv.tile([P, 256], F32, tag="conv")
                    for kc in range(2):
                        mm(ptv[0:cnt, :],
                           y_b[:, kc, mcv * P: mcv * P + cnt],
                           wvt[:, kc, :], start=(kc == 0), stop=(kc == 1))
                    nc.vector.tensor_copy(
                        v_aug[0:cnt, mcv, :].rearrange(
                            "p (h c) -> p h c", c=32)[:, :, 1:17],
                        ptv[0:cnt, :].rearrange("p (h d) -> p h d", d=16))

                # per-group attention (4 heads per group, col-tiled AV)
                for g in range(4):
                    av = ps_av.tile([P, 1024], F32, tag="av")
                    for nh in range(2):
                        mm(av[:, nh * 512: nh * 512 + NH2],
                           zcb[:], zrb[:], start=True, stop=False,
                           skip_group_check=True)
                    for j in range(4):
                        h = 4 * g + j
                        ch, off = h // 4, 32 * (h % 4)
                        for mcv in range(5):
                            cnt = MCNTS[mcv]
                            sp = ps_conv.tile([P, 1024], F32, tag="conv")
                            for nh in range(2):
                                mm(sp[0:cnt, nh * 512: nh * 512 + NH2],
                                   k_pad[off:off + 32, ch,
                                         mcv * P: mcv * P + cnt],
                                   q_pad[off:off + 32, ch,
                                         nh * NH2:(nh + 1) * NH2],
                                   start=True, stop=True,
                                   tile_position=(off, 0))
                            est = est_pool.tile([P, 2, NH2], BF16, tag="est")
                            nc.scalar.activation(
                                est[0:cnt, :, :],
                                sp[0:cnt, :].rearrange(
                                    "p (a f) -> p a f", f=512)[:, :, 0:NH2],
                                AF.Exp, scale=0.25)
                            for nh in range(2):
                                mm(av[off:off + 32,
                                      nh * 512: nh * 512 + NH2],
                                   v_aug[0:cnt, mcv, 32 * h:32 * h + 32],
                                   est[0:cnt, nh, :],
                                   start=False, stop=(mcv == 4),
                                   tile_position=(0, off),
                                   skip_group_check=True)
                    # normalize group: denom rows at 32j (ones-first layout)
                    av_sb = avsb_pool.tile([P, 2, NH2], F32, tag="avsb")
                    nc.vector.tensor_copy(av_sb[:], h3(av))
                    den4 = small.tile([4, 2, NH2], F32, tag="den4")
                    nc.sync.dma_start(den4[:], av_sb[0:P:32, :, :])
                    rec4 = small.tile([4, 2, NH2], BF16, tag="rec4")
                    nc.vector.reciprocal(rec4[:], den4[:])
                    for nh in range(2):
                        rb = ps_av.tile([P, NH2], F32, tag="av")
                        mm(rb[:], e4t[:], rec4[:, nh, :],
                           start=True, stop=True)
                        nc.vector.tensor_tensor(
                            attn_pad[:, g, nh * NH2:(nh + 1) * NH2],
                            av_sb[:, nh, :], rb[:], op=OP.mult)

                # aproj + pe(depthwise) + shortcut -> a_psa
                a_psa = sb.tile([P, 2, N], BF16, bufs=4)
                S["a_psa"] = a_psa
                for mc in range(2):
                    pt = ps_conv.tile([P, 1024], F32, tag="conv")
                    for nh in range(2):
                        for kc in range(4):
                            mm(pt[:, nh * 512: nh * 512 + NH2],
                               wat[:, kc, mc * P:(mc + 1) * P],
                               attn_pad[:, kc, nh * NH2:(nh + 1) * NH2],
                               start=(kc == 0), stop=False)
                        for t in range(9):
                            u, v = t // 3, t % 3
                            win = b_pad[:, mc, :].rearrange(
                                "p (h w) -> p h w", w=26)[
                                :, u + nh * 12: u + nh * 12 + 12, v: v + 24]
                            mm(pt[:, nh * 512: nh * 512 + NH2].rearrange(
                                   "p (h w) -> p h w", w=24),
                               diag_sb[:, mc * 9 + t, :], win,
                               start=False, stop=(t == 8))
                    nc.vector.tensor_tensor(
                        a_psa[:, mc, :].rearrange("p (a f) -> p a f", f=NH2),
                        h3(pt),
                        y_b[:, mc, :].rearrange("p (a f) -> p a f", f=NH2),
                        op=OP.add)

            def s1b(S, payA, payB, sA, sB):
                """ffn1 convs + BN2 stats."""
                a_psa = S["a_psa"]
                h_ffn = sb.tile([P, 4, N], BF16, bufs=3)
                S["h_ffn"] = h_ffn
                st = small.tile([P, 4, 2], F32, tag="st2")
                for mc in range(4):
                    pt = ps_conv.tile([P, 1024], F32, tag="conv")
                    for nh in range(2):
                        for kc in range(2):
                            mm(pt[:, nh * 512: nh * 512 + NH2],
                               wf1t[:, kc, mc * P:(mc + 1) * P],
                               a_psa[:, kc, nh * NH2:(nh + 1) * NH2],
                               start=(kc == 0), stop=(kc == 1))
                    evac_stats(pt, h_ffn[:, mc, :].rearrange(
                        "p (a f) -> p a f", f=NH2), st, mc)
                stats_pay(payA, 4, 4, st)

            def s2(S, payA, payB, sA, sB):
                """BN2 apply + ffn2 convs + BN3 stats."""
                h_ffn = S["h_ffn"]
                r2, nb2 = bn_coeff(sA, 4, 4, tag=2)
                for mc in range(4):
                    buf = h_ffn[:, mc, :]
                    nc.scalar.activation(buf, buf, AF.Relu,
                                         bias=nb2[:, mc:mc + 1],
                                         scale=r2[:, mc:mc + 1])
                f_tmp = sb.tile([P, 2, N], BF16, bufs=3)
                S["f_tmp"] = f_tmp
                st = small.tile([P, 2, 2], F32, tag="st3")
                for mc in range(2):
                    pt = ps_tail.tile([P, 1024], F32, tag="tail")
                    for nh in range(2):
                        for kc in range(4):
                            mm(pt[:, nh * 512: nh * 512 + NH2],
                               wf2t[:, kc, mc * P:(mc + 1) * P],
                               h_ffn[:, kc, nh * NH2:(nh + 1) * NH2],
                               start=(kc == 0), stop=(kc == 3))
                    evac_stats(pt, f_tmp[:, mc, :].rearrange(
                        "p (a f) -> p a f", f=NH2), st, mc)
                stats_pay(payA, 8, 2, st)

            def s3(S, payA, payB, sA, sB):
                """BN3 apply + psa shortcut + ec conv + BN4 stats."""
                f_tmp, a_psa = S["f_tmp"], S["a_psa"]
                r3, nb3 = bn_coeff(sA, 8, 2, tag=3)
                p_sb = sb.tile([P, 2, N], BF16, bufs=3)
                S["p_sb"] = p_sb
                for mc in range(2):
                    buf = f_tmp[:, mc, :]
                    nc.scalar.activation(buf, buf, AF.Identity,
                                         bias=nb3[:, mc:mc + 1],
                                         scale=r3[:, mc:mc + 1])
                    nc.vector.tensor_tensor(p_sb[:, mc, :], f_tmp[:, mc, :],
                                            a_psa[:, mc, :], op=OP.add)
                e_sb = sb.tile([P, N], F32, bufs=2)
                S["e_sb"] = e_sb
                st = small.tile([P, 1, 2], F32, tag="st4")
                ec_pt = ps_tail.tile([P, 1024], F32, tag="tail")
                for nh in range(2):
                    for kc in range(2):
                        mm(ec_pt[:, nh * 512: nh * 512 + NH2],
                           wect[:, kc, :],
                           p_sb[:, kc, nh * NH2:(nh + 1) * NH2],
                           start=(kc == 0), stop=(kc == 1))
                evac_stats(ec_pt, e_sb[:].rearrange("p (a f) -> p a f",
                                                    f=NH2), st, 0)
                stats_pay(payA, 10, 1, st)

            def s4(S, payA, payB, sA, sB):
                """BN4 apply + sigmoid gate + mproj convs + gated BN5 stats."""
                e_sb = S["e_sb"]
                r4, nb4 = bn_coeff(sA, 10, 1, tag=4)
                nc.scalar.activation(e_sb[:], e_sb[:], AF.Relu,
                                     bias=nb4[:, 0:1], scale=r4[:, 0:1])
                # gate = sigmoid(sum(e * g) / sqrt(128*576))
                acc_e = small.tile([P, 1], F32, tag="acc_e")
                nc.scalar.activation(junk_sq[:, 0:N], e_sb[:], AF.Copy,
                                     scale=gvec[:], accum_out=acc_e[:])
                gd_ps = ps_tail.tile([1, 1], F32, tag="tail")
                nc.tensor.matmul(gd_ps[:], ones_col[:], acc_e[:],
                                 start=True, stop=True)
                sg = small.tile([1, 1], F32, tag="sg")
                nc.scalar.activation(sg[:], gd_ps[:], AF.Exp,
                                     scale=-1.0 / float(np.sqrt(128.0 * N)))
                sg1 = small.tile([1, 1], F32, tag="sg1")
                nc.vector.tensor_scalar(sg1[:], sg[:], 1.0, None, op0=OP.add)
                grec = small.tile([1, 1], F32, tag="grec")
                nc.vector.reciprocal(grec[:], sg1[:])
                gb_ps = ps_tail.tile([P, 1], F32, tag="tail")
                nc.tensor.matmul(gb_ps[:], ones_row[:], grec[:],
                                 start=True, stop=True)
                gb = small.tile([P, 1], F32, tag="gb")
                nc.vector.tensor_copy(gb[:], gb_ps[:])
                gb2 = small.tile([P, 1], F32, tag="gb2")
                nc.vector.tensor_tensor(gb2[:], gb[:], gb[:], op=OP.mult)
                S["gb"], S["gb2"] = gb, gb2

                # e_pad + mproj (gate folded into BN via gated stats)
                e_pad = sb.tile([P, 676], BF16, bufs=2)
                nc.vector.memset(e_pad[:], 0.0)
                nc.vector.tensor_copy(
                    e_pad[:].rearrange("p (h w) -> p h w", w=26)[:, 1:25, 1:25],
                    e_sb[:].rearrange("p (h w) -> p h w", w=24))
                m_sb = sb.tile([P, 2, N], BF16, bufs=3)
                S["m_sb"] = m_sb
                st = small.tile([P, 2, 2], F32, tag="st5")
                for mc in range(2):
                    pt = ps_tail.tile([P, 1024], F32, tag="tail")
                    for nh in range(2):
                        for t in range(9):
                            u, v = t // 3, t % 3
                            win = e_pad[:].rearrange("p (h w) -> p h w",
                                                     w=26)[
                                :, u + nh * 12: u + nh * 12 + 12, v: v + 24]
                            mm(pt[:, nh * 512: nh * 512 + NH2].rearrange(
                                   "p (h w) -> p h w", w=24),
                               wmpt[:, t, mc * P:(mc + 1) * P], win,
                               start=(t == 0), stop=(t == 8))
                    evac_stats(pt, m_sb[:, mc, :].rearrange(
                        "p (a f) -> p a f", f=NH2), st, mc)
                stats_pay(payB, 0, 2, st, gate2=(gb, gb2))

            def s5(S, payA, payB, sA, sB):
                """BN5 apply (gated) + full cv2 convs + BN6 stats."""
                m_sb, gb = S["m_sb"], S["gb"]
                r5, nb5 = bn_coeff(sB, 0, 2, tag=5)
                r5g = small.tile([P, 2], F32, tag="r5g")
                nc.vector.tensor_scalar(r5g[:], r5[:], gb[:], None,
                                        op0=OP.mult)
                for mc in range(2):
                    buf = m_sb[:, mc, :]
                    nc.scalar.activation(buf, buf, AF.Relu,
                                         bias=nb5[:, mc:mc + 1],
                                         scale=r5g[:, mc:mc + 1])
                y_a, y_b, p_sb = S["y_a"], S["y_b"], S["p_sb"]
                cat2 = [y_a[:, 0, :], y_a[:, 1, :], y_b[:, 0, :], y_b[:, 1, :],
                        p_sb[:, 0, :], p_sb[:, 1, :],
                        m_sb[:, 0, :], m_sb[:, 1, :]]
                out_sb = sb.tile([P, 4, N], BF16, bufs=3)
                S["out_sb"] = out_sb
                st = small.tile([P, 4, 2], F32, tag="st6")
                for mc in range(4):
                    pt = ps_tail.tile([P, 1024], F32, tag="tail")
                    for nh in range(2):
                        for kc in range(8):
                            mm(pt[:, nh * 512: nh * 512 + NH2],
                               w2t[:, kc, mc * P:(mc + 1) * P],
                               cat2[kc][:, nh * NH2:(nh + 1) * NH2],
                               start=(kc == 0), stop=(kc == 7))
                    evac_stats(pt, out_sb[:, mc, :].rearrange(
                        "p (a f) -> p a f", f=NH2), st, mc)
                stats_pay(payB, 2, 4, st)

            def s6(S, payA, payB, sA, sB):
                """BN6 apply + output DMA."""
                out_sb = S["out_sb"]
                r6, nb6 = bn_coeff(sB, 2, 4, tag=6)
                for mc in range(4):
                    buf = out_sb[:, mc, :]
                    nc.scalar.activation(buf, buf, AF.Relu,
                                         bias=nb6[:, mc:mc + 1],
                                         scale=r6[:, mc:mc + 1])
                    nc.sync.dma_start(
                        out_d[:].rearrange("(a p) f -> p a f", p=P)[:, mc, :],
                        buf)
                S.clear()

            stages = [s0, s1a, s1b, s2, s3, s4, s5, s6]
            a_producers = [0, 2, 3, 4]     # s0, s1b, s2, s3 fill payA
            b_producers = [1, 5, 6, 7]     # s1a (no pay), s4, s5, s6

            def do_coll(pay, nch, tag):
                bin_ = dram.tile([P, nch, 2], F32, tag=f"ccin{tag}",
                                 name=f"bin{tag}")
                bout = dram.tile([NCORES, P, nch, 2], F32, tag=f"ccout{tag}",
                                 name=f"bout{tag}")
                nc.sync.dma_start(bin_[:], pay[:])
                nc.gpsimd.collective_compute(
                    "AllGather", OP.bypass,
                    replica_groups=[list(range(NCORES))],
                    ins=[bin_[:].opt()], outs=[bout[:].opt()])
                gat = small.tile([P, NCORES, nch, 2], F32, tag=f"gat{tag}",
                                 name=f"gat{tag}")
                for rk in range(NCORES):
                    nc.sync.dma_start(gat[:, rk, :, :], bout[rk])
                sums = small.tile([P, nch, 2, 1], F32, tag=f"sums{tag}",
                                  name=f"sums{tag}")
                nc.vector.reduce_sum(sums[:],
                                     gat[:].transpose([0, 2, 3, 1]),
                                     axis=mybir.AxisListType.X)
                return sums[:, :, :, 0]

            # ===================== pipeline driver =====================
            states = [dict() for _ in range(repeat)]
            sums_of_step = {}
            for t in range(repeat + NST - 1):
                payA = small.tile([P, PAYA, 2], F32, tag="payA")
                payB = small.tile([P, PAYB, 2], F32, tag="payB")
                nc.vector.memset(payA[:], 0.0)
                nc.vector.memset(payB[:], 0.0)
                sA, sB = sums_of_step.pop(t - 1, (None, None))
                for s in a_producers:
                    k = t - s
                    if 0 <= k < repeat:
                        stages[s](states[k], payA, payB, sA, sB)
                cA = payA if no_coll else do_coll(payA, PAYA, "A")
                for s in b_producers:
                    k = t - s
                    if 0 <= k < repeat:
                        stages[s](states[k], payA, payB, sA, sB)
                cB = payB if no_coll else do_coll(payB, PAYB, "B")
                sums_of_step[t] = (cA, cB)

    nc.compile()
    return nc


def get_nc(repeat=1, no_coll=False, ndev=NCORES):
    key = f"nc{repeat}_{no_coll}_{ndev}"
    if key not in _CACHE:
        _CACHE[key] = _build(repeat, no_coll, ndev)
    return _CACHE[key]


def host_prep(inputs):
    """Fold modulus*cos(phase) weights, build padded/transposed layouts and
    the 8 per-core input maps."""
    def w(m, p):
        return (np.asarray(inputs[m], np.float32)
                * np.cos(np.asarray(inputs[p], np.float32)))

    x = np.asarray(inputs["x"], np.float32)          # (2, 512, 4, 24, 24)
    guide = np.asarray(inputs["guide"], np.float32)  # (2, 512)
    B, C1, Qd = x.shape[0], x.shape[1], x.shape[2]

    w1 = w("cv1_m", "cv1_p")[:, :, 0, 0]             # (512, 512) [co, ci]
    wqkv = w("qkv_m", "qkv_p")[:, :, 0, 0]           # (768, 256)
    wq, wk, wv = wqkv[0:256], wqkv[256:512], wqkv[512:768]
    wa = w("aproj_m", "aproj_p")[:, :, 0, 0]         # (256, 256)
    pe = w("pe_m", "pe_p")[:, 0, :, :].reshape(256, 9)
    wf1 = w("ffn1_m", "ffn1_p")[:, :, 0, 0]          # (512, 256)
    wf2 = w("ffn2_m", "ffn2_p")[:, :, 0, 0]          # (256, 512)
    wec = w("ec_m", "ec_p")[:, :, 0, 0]              # (128, 256)
    wmp = w("mproj_m", "mproj_p")                    # (256, 128, 3, 3)
    w2 = w("cv2_m", "cv2_p")[:, :, 0, 0]             # (512, 1024)
    gl_w = np.asarray(inputs["gl_w"], np.float32)
    gl_b = np.asarray(inputs["gl_b"], np.float32)
    gfull = guide @ gl_w.T + gl_b                    # (2, 128)

    # q/k head-padded [ci, co'=512]: col 32h+d = W[16h+d, ci], d<16
    def pad_qk(wm):
        out = np.zeros((256, 512), np.float32)
        for h in range(16):
            out[:, 32 * h:32 * h + 16] = wm[16 * h:16 * h + 16, :].T
        return out

    wqt = pad_qk(wq)
    wkt = pad_qk(wk)
    # aproj with padded ci' (ones-first attn layout: channel' 32h+1+d)
    wat_pad = np.zeros((512, 256), np.float32)
    for h in range(16):
        wat_pad[32 * h + 1:32 * h + 17, :] = wa[:, 16 * h:16 * h + 16].T
    e4 = np.zeros((4, 128), np.float32)
    for j in range(4):
        e4[j, 32 * j + 1:32 * j + 17] = 1.0

    import ml_dtypes
    bf16 = ml_dtypes.bfloat16
    shared = {
        "w1t": np.ascontiguousarray(w1.T).astype(bf16),
        "wqt": wqt.astype(bf16), "wkt": wkt.astype(bf16),
        "wvt": np.ascontiguousarray(wv.T).astype(bf16),
        "wat_pad": wat_pad.astype(bf16),
        "pe_w": pe,
        "wf1t": np.ascontiguousarray(wf1.T).astype(bf16),
        "wf2t": np.ascontiguousarray(wf2.T).astype(bf16),
        "wect": np.ascontiguousarray(wec.T).astype(bf16),
        "wmpt": np.ascontiguousarray(
            wmp.transpose(2, 3, 1, 0).reshape(9, 128, 256)).astype(bf16),
        "w2t": np.ascontiguousarray(w2.T).astype(bf16),
        "id128": np.eye(128, dtype=np.float32),
        "e4": e4.astype(bf16),
    }
    in_maps = []
    for core in range(NCORES):
        b, q = core // Qd, core % Qd
        m = dict(shared)
        m["x_s"] = np.ascontiguousarray(x[b, :, q].reshape(C1, N)).astype(bf16)
        m["gvec"] = np.ascontiguousarray(gfull[b].reshape(P, 1))
        in_maps.append(m)
    return in_maps, (B, Qd)


def get_runner(repeat=1, no_coll=False):
    """Cached sharded jitted executable over the 8 axon cores, mirroring
    bass2jax.run_bass_via_pjrt (which re-traces on every call)."""
    rkey = f"runner{repeat}_{no_coll}"
    if rkey in _CACHE:
        return _CACHE[rkey]
    import jax
    import numpy as _np
    from jax.sharding import Mesh, PartitionSpec
    from jax.experimental.shard_map import shard_map
    import concourse.mybir as mybir
    from concourse.bass2jax import (_bass_exec_p, partition_id_tensor,
                                    install_neuronx_cc_hook)

    nc = get_nc(repeat, no_coll)
    install_neuronx_cc_hook()
    partition_name = (nc.partition_id_tensor.name
                      if nc.partition_id_tensor else None)
    in_names, out_names, out_avals, zero_outs = [], [], [], []
    for alloc in nc.m.functions[0].allocations:
        if not isinstance(alloc, mybir.MemoryLocationSet):
            continue
        name = alloc.memorylocations[0].name
        if alloc.kind == "ExternalInput":
            if name != partition_name:
                in_names.append(name)
        elif alloc.kind == "ExternalOutput":
            shape = tuple(alloc.tensor_shape)
            dtype = mybir.dt.np(alloc.dtype)
            out_names.append(name)
            out_avals.append(jax.core.ShapedArray(shape, dtype))
            zero_outs.append(_np.zeros(shape, dtype))
    n_params, n_outs = len(in_names), len(out_avals)
    all_in_names = list(in_names) + list(out_names)
    if partition_name is not None:
        all_in_names.append(partition_name)
    donate = tuple(range(n_params, n_params + n_outs))

    def _body(*args):
        operands = list(args)
        if partition_name is not None:
            operands.append(partition_id_tensor())
        outs = _bass_exec_p.bind(
            *operands,
            out_avals=tuple(out_avals),
            in_names=tuple(all_in_names),
            out_names=tuple(out_names),
            lowering_input_output_aliases=(),
            sim_require_finite=True,
            sim_require_nnan=True,
            nc=nc,
        )
        return tuple(outs)

    devices = jax.devices()[:NCORES]
    mesh = Mesh(_np.asarray(devices), ("core",))
    in_specs = (PartitionSpec("core"),) * (n_params + n_outs)
    out_specs = (PartitionSpec("core"),) * n_outs
    sharded = jax.jit(
        shard_map(_body, mesh=mesh, in_specs=in_specs, out_specs=out_specs,
                  check_rep=False),
        donate_argnums=donate, keep_unused=True)
    runner = {
        "fn": sharded, "mesh": mesh, "in_names": in_names,
        "out_names": out_names, "out_avals": out_avals,
        "zero_outs": zero_outs, "n_params": n_params,
    }
    _CACHE[rkey] = runner
    return runner


def run_cores(in_maps):
    import numpy as _np
    r = get_runner()
    concat_in = [
        _np.concatenate([_np.asarray(in_maps[c][name])[None]
                         for c in range(NCORES)], axis=0).reshape(
            NCORES * in_maps[0][name].shape[0], *in_maps[0][name].shape[1:])
        for name in r["in_names"]]
    concat_zeros = [
        _np.zeros((NCORES * z.shape[0], *z.shape[1:]), z.dtype)
        for z in r["zero_outs"]]
    out_arrs = r["fn"](*concat_in, *concat_zeros)
    outs = []
    for c in range(NCORES):
        outs.append({
            name: _np.asarray(out_arrs[i]).reshape(
                NCORES, *r["out_avals"][i].shape)[c]
            for i, name in enumerate(r["out_names"])})
    return outs


def kernel(**inputs):
    in_maps, (B, Qd) = host_prep(inputs)
    results = run_cores(in_maps)
    out = np.zeros((B, 512, Qd, 24, 24), np.float32)
    for core in range(NCORES):
        b, q = core // Qd, core % Qd
        out[b, :, q] = np.asarray(results[core]["out"],
                                  np.float32).reshape(512, 24, 24)
    return out
